# revision 5
# baseline (speedup 1.0000x reference)
"""Distributed flash-decoding attention kernel for 8 TRN2 NeuronCores.

Problem: B=1024 new tokens attend over a 32768-row KV cache plus the new
block (causal within the block). Sequence-parallel sharding: each core
handles 4096 cache rows + 128 new rows (4224 keys), computes partial
exp-scores @ V_aug (V with a ones column so the softmax normalizer comes
out of the same matmul), then a ReduceScatter combines partials and each
core emits its 128-query slice of the output.
"""

import os
import sys

import numpy as np

for _p in ("/opt/trn_rl_repo",):
    if os.path.isdir(_p) and _p not in sys.path:
        sys.path.insert(0, _p)

import ml_dtypes  # noqa: E402
import concourse.bacc as bacc  # noqa: E402
import concourse.mybir as mybir  # noqa: E402
import concourse.tile as tile  # noqa: E402
from concourse.bass_utils import run_bass_kernel_spmd  # noqa: E402

N_CORES = 8
B, S, DK, DV = 1024, 32768, 128, 128
S_SH = S // N_CORES  # 4096 cache rows per core
B_SH = B // N_CORES  # 128 new rows per core
NKEY = S_SH + B_SH  # 4224 keys per core
NT = NKEY // 128  # 33 key tiles
DVA = DV + 4  # 132: dv cols + l col (128) + 3 pad
QHW = 512  # q half width
F32 = mybir.dt.float32
F32R = mybir.dt.float32r
BF16 = mybir.dt.bfloat16
I32 = mybir.dt.int32

KT_CH = 11  # kt DMA chunks (3 key tiles each)
KT_W = (NT // KT_CH) * 128  # 384
VA_CH = 3  # vaug DMA chunks (11 key tiles each)
VA_T = NT // VA_CH  # 11


def _declare_io(nc):
    return dict(
        kt=nc.dram_tensor("kt", [128, NKEY], F32R, kind="ExternalInput"),
        qt=nc.dram_tensor("qt", [128, B], F32R, kind="ExternalInput"),
        vaug=nc.dram_tensor(
            "vaug", [NT, 128, DVA], BF16, kind="ExternalInput"
        ),
        thr=nc.dram_tensor("thr", [128, 1], F32, kind="ExternalInput"),
        out=nc.dram_tensor("out", [B_SH, DV], F32, kind="ExternalOutput"),
    )


def _emit_mask(nc, pmisc, th_d):
    """mask01[p, f] = 1.0 if query f >= (c*128 + p) else 0.0 (bf16)."""
    iota_i = pmisc.tile([128, B], I32, name="iota_i", tag="iota_i")
    nc.gpsimd.iota(iota_i[:], pattern=[[1, B]], base=0, channel_multiplier=0)
    iota_f = pmisc.tile([128, B], F32, name="iota_f", tag="iota_f")
    nc.vector.tensor_copy(iota_f[:], iota_i[:])
    thr_sb = pmisc.tile([128, 1], F32, name="thr", tag="thr")
    nc.sync.dma_start(thr_sb[:], th_d[:])
    mask01 = pmisc.tile([128, B], BF16, name="mask", tag="mask")
    nc.vector.tensor_scalar(
        out=mask01[:],
        in0=iota_f[:],
        scalar1=thr_sb[:],
        scalar2=None,
        op0=mybir.AluOpType.is_ge,
    )
    return mask01


def _emit_body(nc, pools, io, mask01, part):
    """Loads + compute for one pass; writes the [B, DVA] partial to `part`."""
    pkt, pqt, pva, pexp, ps_s, ps_oa = (
        pools["pkt"],
        pools["pqt"],
        pools["pva"],
        pools["pexp"],
        pools["ps_s"],
        pools["ps_oa"],
    )
    kt_sbs = []
    for i in range(KT_CH):
        t = pkt.tile([128, KT_W], F32R, name=f"kt{i}", tag=f"kt{i}")
        nc.sync.dma_start(t[:], io["kt"][:, i * KT_W : (i + 1) * KT_W])
        kt_sbs.append(t)
    qt_sbs = []
    for h in range(2):
        t = pqt.tile([128, QHW], F32R, name=f"qt{h}", tag=f"qt{h}")
        nc.sync.dma_start(t[:], io["qt"][:, h * QHW : (h + 1) * QHW])
        qt_sbs.append(t)
    va_sbs = []
    for i in range(VA_CH):
        t = pva.tile([128, VA_T, DVA], BF16, name=f"va{i}", tag=f"va{i}")
        nc.sync.dma_start(
            t[:],
            io["vaug"][i * VA_T : (i + 1) * VA_T, :, :].rearrange(
                "t p d -> p t d"
            ),
        )
        va_sbs.append(t)

    for qh in range(2):
        oa = [ps_oa.tile([128, DVA], F32, name=f"oa{qs}", tag=f"oa{qs}") for qs in range(4)]
        for t in range(NT):
            kt_ap = kt_sbs[t // 3][:, (t % 3) * 128 : (t % 3 + 1) * 128]
            s_ps = ps_s.tile([128, QHW], F32, name="s", tag="s")
            nc.tensor.matmul(
                s_ps[:],
                kt_ap,
                qt_sbs[qh][:],
                start=True,
                stop=True,
            )
            e_sb = pexp.tile([128, QHW], BF16, name="e", tag="e")
            nc.scalar.activation(
                e_sb[:], s_ps[:], mybir.ActivationFunctionType.Exp
            )
            if t == NT - 1:
                e_m = pexp.tile([128, QHW], BF16, name="em", tag="em")
                nc.vector.tensor_tensor(
                    out=e_m[:],
                    in0=e_sb[:],
                    in1=mask01[:, qh * QHW : (qh + 1) * QHW],
                    op=mybir.AluOpType.mult,
                )
                e_sb = e_m
            va_ap = va_sbs[t // VA_T][:, t % VA_T, :]
            for qs in range(4):
                nc.tensor.matmul(
                    oa[qs][:],
                    e_sb[:, qs * 128 : (qs + 1) * 128],
                    va_ap,
                    start=(t == 0),
                    stop=(t == NT - 1),
                )
        for qs in range(4):
            oa_sb = pexp.tile([128, DVA], F32, name="oasb", tag="oasb")
            nc.vector.tensor_copy(oa_sb[:], oa[qs][:])
            r0 = (qh * 4 + qs) * 128
            nc.sync.dma_start(part[r0 : r0 + 128, :], oa_sb[:])


def _emit_combine(nc, pep, part, red, out_d):
    nc.gpsimd.collective_compute(
        "ReduceScatter",
        mybir.AluOpType.add,
        replica_groups=[list(range(N_CORES))],
        ins=[part.opt()],
        outs=[red.opt()],
    )
    red_sb = pep.tile([B_SH, DVA], F32, name="red_sb", tag="red_sb")
    nc.sync.dma_start(red_sb[:], red[:])
    linv = pep.tile([B_SH, 1], F32, name="linv", tag="linv")
    nc.vector.reciprocal(linv[:], red_sb[:, DV : DV + 1])
    out_sb = pep.tile([B_SH, DV], F32, name="out_sb", tag="out_sb")
    nc.vector.tensor_scalar_mul(out_sb[:], red_sb[:, :DV], linv[:])
    nc.sync.dma_start(out_d[:], out_sb[:])


def build_nc(loop_iters: int | None = None):
    """loop_iters=None: real kernel (compute + ReduceScatter + epilogue).
    loop_iters=N: timing variant — compute body inside tc.For_i(0, N, 1),
    no collective (collectives can't sit inside control flow)."""
    nc = bacc.Bacc(
        "TRN2", target_bir_lowering=False, debug=False, num_devices=N_CORES
    )
    io = _declare_io(nc)
    with tile.TileContext(nc) as tc:
        with (
            tc.tile_pool(name="pkt", bufs=2) as pkt,
            tc.tile_pool(name="pqt", bufs=2) as pqt,
            tc.tile_pool(name="pva", bufs=2) as pva,
            tc.tile_pool(name="pexp", bufs=4) as pexp,
            tc.tile_pool(name="pmisc", bufs=1) as pmisc,
            tc.tile_pool(name="pep", bufs=2) as pep,
            tc.tile_pool(name="ps_s", bufs=3, space="PSUM") as ps_s,
            tc.tile_pool(name="ps_oa", bufs=1, space="PSUM") as ps_oa,
            tc.tile_pool(name="pdram", bufs=2, space="DRAM") as pdram,
        ):
            pools = dict(
                pkt=pkt, pqt=pqt, pva=pva, pexp=pexp, ps_s=ps_s, ps_oa=ps_oa
            )
            mask01 = _emit_mask(nc, pmisc, io["thr"])
            if loop_iters is None:
                part = pdram.tile([B, DVA], F32, name="part", tag="part")
                red = pdram.tile(
                    [B_SH, DVA], F32, name="red", tag="red"
                )
                _emit_body(nc, pools, io, mask01, part)
                _emit_combine(nc, pep, part, red, io["out"])
            else:
                part = pdram.tile([B, DVA], F32, name="part", tag="part")
                with tc.For_i(0, loop_iters, 1):
                    _emit_body(nc, pools, io, mask01, part)
                # dummy output so the NEFF has a valid ExternalOutput write
                out_sb = pep.tile([B_SH, DV], F32, name="out_sb", tag="out_sb")
                nc.vector.memset(out_sb[:], 0.0)
                nc.sync.dma_start(io["out"][:], out_sb[:])
    nc.compile()
    return nc


_CACHE: dict = {}


def _get_nc():
    if "nc" not in _CACHE:
        _CACHE["nc"] = build_nc()
    return _CACHE["nc"]


def make_in_maps(q, k, v, K_cache, V_cache):
    q = np.asarray(q, np.float32)
    k = np.asarray(k, np.float32)
    v = np.asarray(v, np.float32)
    K_cache = np.asarray(K_cache, np.float32)
    V_cache = np.asarray(V_cache, np.float32)

    scale = 1.0 / np.sqrt(np.float32(DK))
    qt = np.ascontiguousarray((q * scale).T)  # [128, 1024]

    in_maps = []
    for c in range(N_CORES):
        Ksh = np.concatenate(
            [K_cache[c * S_SH : (c + 1) * S_SH], k[c * B_SH : (c + 1) * B_SH]],
            axis=0,
        )  # [4224, 128]
        kt = np.ascontiguousarray(Ksh.T)  # [128, 4224]
        Vsh = np.concatenate(
            [V_cache[c * S_SH : (c + 1) * S_SH], v[c * B_SH : (c + 1) * B_SH]],
            axis=0,
        )
        va = np.zeros((NKEY, DVA), np.float32)
        va[:, :DV] = Vsh
        va[:, DV] = 1.0
        va = va.reshape(NT, 128, DVA).astype(ml_dtypes.bfloat16)
        thr = (c * B_SH + np.arange(128, dtype=np.float32)).reshape(128, 1)
        in_maps.append({"kt": kt, "qt": qt, "vaug": va, "thr": thr})
    return in_maps


def kernel(q, k, v, K_cache, V_cache):
    in_maps = make_in_maps(q, k, v, K_cache, V_cache)
    res = run_bass_kernel_spmd(
        _get_nc(), in_maps, core_ids=list(range(N_CORES))
    )
    out = np.concatenate(
        [res.results[c]["out"] for c in range(N_CORES)], axis=0
    )
    return np.ascontiguousarray(out, dtype=np.float32)


# revision 20
# speedup vs baseline: 51.1413x; 51.1413x over previous
"""Distributed flash-decoding attention kernel for 8 TRN2 NeuronCores.

Problem: B=1024 new tokens attend over a 32768-row KV cache plus the new
block (causal within the block). Sequence-parallel sharding: each core
handles 4096 cache rows + 128 new rows (4224 keys), computes partial
exp-scores @ V_aug (V with a ones column so the softmax normalizer comes
out of the same matmul), then a ReduceScatter combines partials and each
core emits its 128-query slice of the output.
"""

import os
import sys

import numpy as np

for _p in ("/opt/trn_rl_repo",):
    if os.path.isdir(_p) and _p not in sys.path:
        sys.path.insert(0, _p)

import ml_dtypes  # noqa: E402
import concourse.bacc as bacc  # noqa: E402
import concourse.mybir as mybir  # noqa: E402
import concourse.tile as tile  # noqa: E402
from concourse.bass_utils import run_bass_kernel_spmd  # noqa: E402

N_CORES = 8
B, S, DK, DV = 1024, 32768, 128, 128
S_SH = S // N_CORES  # 4096 cache rows per core
B_SH = B // N_CORES  # 128 new rows per core
NKEY = S_SH + B_SH  # 4224 keys per core
NT = NKEY // 128  # 33 key tiles
DVA = DV + 4  # 132: dv cols + l col (128) + 3 pad
QHW = 512  # q half width
F32 = mybir.dt.float32
F32R = mybir.dt.float32r
BF16 = mybir.dt.bfloat16
I32 = mybir.dt.int32

KT_CH = 11  # kt DMA chunks (3 key tiles each)
KT_W = (NT // KT_CH) * 128  # 384
VA_CH = 3  # vaug DMA chunks (11 key tiles each)
VA_T = NT // VA_CH  # 11


def _declare_io(nc):
    return dict(
        kt=nc.dram_tensor("kt", [128, NKEY], F32R, kind="ExternalInput"),
        qt=nc.dram_tensor("qt", [128, B], F32R, kind="ExternalInput"),
        vaug=nc.dram_tensor(
            "vaug", [NT, 128, DVA], BF16, kind="ExternalInput"
        ),
        thr=nc.dram_tensor("thr", [128, 1], F32, kind="ExternalInput"),
        out=nc.dram_tensor("out", [B_SH, DV], F32, kind="ExternalOutput"),
    )


def _emit_mask(nc, pmisc, th_d):
    """mask01[p, f] = 1.0 if query f >= (c*128 + p) else 0.0 (bf16)."""
    iota_i = pmisc.tile([128, B], I32, name="iota_i", tag="iota_i")
    nc.gpsimd.iota(iota_i[:], pattern=[[1, B]], base=0, channel_multiplier=0)
    iota_f = pmisc.tile([128, B], F32, name="iota_f", tag="iota_f")
    nc.vector.tensor_copy(iota_f[:], iota_i[:])
    thr_sb = pmisc.tile([128, 1], F32, name="thr", tag="thr")
    nc.sync.dma_start(thr_sb[:], th_d[:])
    mask01 = pmisc.tile([128, B], BF16, name="mask", tag="mask")
    nc.vector.tensor_scalar(
        out=mask01[:],
        in0=iota_f[:],
        scalar1=thr_sb[:],
        scalar2=None,
        op0=mybir.AluOpType.is_ge,
    )
    return mask01


def _emit_body(nc, pools, io, mask01, part, stage=4):
    """Loads + compute for one pass; writes the [B, DVA] partial to `part`.
    stage: 1=DMA, 2=+scores, 3=+exp, 4=full; 5=no-l, 6=l-only (timing)."""
    pkt, pqt, pva, pexp, ps_s = (
        pools["pkt"],
        pools["pqt"],
        pools["pva"],
        pools["pexp"],
        pools["ps_s"],
    )
    if stage < 1:
        return
    # Two HWDGE rings in parallel: kt + qt0 on the SP ring (nc.sync),
    # vaug + qt1 on the ACT ring (nc.scalar). First chunks of each stream
    # are small so the pipeline starts early.
    qt_sbs = []
    t0 = pqt.tile([128, QHW], F32R, name="qt0", tag="qt0")
    nc.sync.dma_start(t0[:], io["qt"][:, 0:QHW])
    qt_sbs.append(t0)
    va_sbs = []  # (first_tile_idx, n_tiles, tile)
    va_chunks = [(0, 3), (3, 15), (18, 15)]
    fi, n = va_chunks[0]
    va_t = pva.tile([128, n, DVA], BF16, name="va0", tag="va0")
    nc.scalar.dma_start(
        va_t[:], io["vaug"][fi : fi + n, :, :].rearrange("t p d -> p t d")
    )
    va_sbs.append((fi, n, va_t))
    t1 = pqt.tile([128, QHW], F32R, name="qt1", tag="qt1")
    nc.scalar.dma_start(t1[:], io["qt"][:, QHW : 2 * QHW])
    qt_sbs.append(t1)
    kt_sbs = []
    for i in range(KT_CH):
        t = pkt.tile([128, KT_W], F32R, name=f"kt{i}", tag=f"kt{i}")
        nc.sync.dma_start(t[:], io["kt"][:, i * KT_W : (i + 1) * KT_W])
        kt_sbs.append(t)
    for ci, (fi, n) in enumerate(va_chunks[1:], start=1):
        va_t = pva.tile([128, n, DVA], BF16, name=f"va{ci}", tag=f"va{ci}")
        nc.scalar.dma_start(
            va_t[:],
            io["vaug"][fi : fi + n, :, :].rearrange("t p d -> p t d"),
        )
        va_sbs.append((fi, n, va_t))

    def va_ap_for(t):
        for fi, n, tile_ in va_sbs:
            if fi <= t < fi + n:
                return tile_[:, t - fi, :]
        raise AssertionError(t)

    if stage < 2:
        return
    # pass A: scores + exp (resident) + PV for q-subtiles 0..3
    # pass B: PV for q-subtiles 4..7 re-reading the resident exp tiles.
    # PV lags scores by one tile so exp(t) never blocks scores(t+1) at the
    # head of the in-order PE queue.
    ps_oa = pools["ps_oa"]
    saved_e = []
    oa = [
        ps_oa.tile([128, DVA], F32, name=f"oaA{qs}", tag=f"oa{qs}")
        for qs in range(4)
    ]

    def pv_a(t, last):
        va_ap = va_ap_for(t)
        for qs in range(4):
            nc.tensor.matmul(
                oa[qs][:],
                saved_e[t][:, qs * 128 : (qs + 1) * 128],
                va_ap,
                start=(t == 0),
                stop=last,
            )

    for t in range(NT):
        kt_ap = kt_sbs[t // 3][:, (t % 3) * 128 : (t % 3 + 1) * 128]
        s_ps = ps_s.tile([128, B], F32, name="s", tag="s")
        for qh in range(2):
            nc.tensor.matmul(
                s_ps[:, qh * QHW : (qh + 1) * QHW],
                kt_ap,
                qt_sbs[qh][:],
                start=True,
                stop=True,
            )
        if stage < 3:
            continue
        e_sb = pexp.tile([128, B], BF16, name="e", tag="e", bufs=NT)
        nc.scalar.activation(
            e_sb[:], s_ps[:], mybir.ActivationFunctionType.Exp
        )
        if t == NT - 1:
            e_m = pexp.tile([128, B], BF16, name="em", tag="em")
            nc.vector.tensor_tensor(
                out=e_m[:],
                in0=e_sb[:],
                in1=mask01[:],
                op=mybir.AluOpType.mult,
            )
            e_sb = e_m
        saved_e.append(e_sb)
        if stage < 4:
            continue
        if t >= 1:
            pv_a(t - 1, last=False)
    if stage < 4:
        return
    pv_a(NT - 1, last=True)
    for qs in range(4):
        oa_sb = pexp.tile([128, DVA], F32, name="oasb", tag="oasb")
        nc.vector.tensor_copy(oa_sb[:], oa[qs][:])
        nc.sync.dma_start(part[qs * 128 : (qs + 1) * 128, :], oa_sb[:])
    oa2 = [
        ps_oa.tile([128, DVA], F32, name=f"oaB{qs}", tag=f"oa{qs}")
        for qs in range(4)
    ]
    for t in range(NT):
        va_ap = va_ap_for(t)
        for qs in range(4):
            nc.tensor.matmul(
                oa2[qs][:],
                saved_e[t][:, (qs + 4) * 128 : (qs + 5) * 128],
                va_ap,
                start=(t == 0),
                stop=(t == NT - 1),
            )
    for qs in range(4):
        oa_sb = pexp.tile([128, DVA], F32, name="oasb2", tag="oasb")
        nc.vector.tensor_copy(oa_sb[:], oa2[qs][:])
        nc.sync.dma_start(part[(qs + 4) * 128 : (qs + 5) * 128, :], oa_sb[:])


def _emit_combine(nc, pep, part, red, out_d):
    nc.gpsimd.collective_compute(
        "ReduceScatter",
        mybir.AluOpType.add,
        replica_groups=[list(range(N_CORES))],
        ins=[part.opt()],
        outs=[red.opt()],
    )
    red_sb = pep.tile([B_SH, DVA], F32, name="red_sb", tag="red_sb")
    nc.sync.dma_start(red_sb[:], red[:])
    linv = pep.tile([B_SH, 1], F32, name="linv", tag="linv")
    nc.vector.reciprocal(linv[:], red_sb[:, DV : DV + 1])
    out_sb = pep.tile([B_SH, DV], F32, name="out_sb", tag="out_sb")
    nc.vector.tensor_scalar_mul(out_sb[:], red_sb[:, :DV], linv[:])
    nc.sync.dma_start(out_d[:], out_sb[:])


def build_nc(loop_iters: int | None = None, stage: int = 4):
    """loop_iters=None: real kernel (compute + ReduceScatter + epilogue).
    loop_iters=N: timing variant — compute body inside tc.For_i(0, N, 1),
    no collective (collectives can't sit inside control flow)."""
    nc = bacc.Bacc(
        "TRN2", target_bir_lowering=False, debug=False, num_devices=N_CORES
    )
    io = _declare_io(nc)
    with tile.TileContext(nc) as tc:
        with (
            tc.tile_pool(name="pkt", bufs=2) as pkt,
            tc.tile_pool(name="pqt", bufs=2) as pqt,
            tc.tile_pool(name="pva", bufs=2) as pva,
            tc.tile_pool(name="pexp", bufs=4) as pexp,
            tc.tile_pool(name="pmisc", bufs=1) as pmisc,
            tc.tile_pool(name="pep", bufs=2) as pep,
            tc.tile_pool(name="ps_s", bufs=2, space="PSUM") as ps_s,
            tc.tile_pool(name="ps_oa", bufs=1, space="PSUM") as ps_oa,
            tc.tile_pool(name="pdram", bufs=2, space="DRAM") as pdram,
        ):
            pools = dict(
                pkt=pkt, pqt=pqt, pva=pva, pexp=pexp, ps_s=ps_s, ps_oa=ps_oa
            )
            mask01 = _emit_mask(nc, pmisc, io["thr"])
            if loop_iters is None:
                part = pdram.tile([B, DVA], F32, name="part", tag="part")
                red = pdram.tile([B_SH, DVA], F32, name="red", tag="red")
                _emit_body(nc, pools, io, mask01, part)
                _emit_combine(nc, pep, part, red, io["out"])
            elif loop_iters == 0:
                # compute-only, single pass, no collective (for TimelineSim)
                part = pdram.tile([B, DVA], F32, name="part", tag="part")
                _emit_body(nc, pools, io, mask01, part)
                out_sb = pep.tile([B_SH, DV], F32, name="out_sb0", tag="out_sb")
                nc.vector.memset(out_sb[:], 0.0)
                nc.sync.dma_start(io["out"][:], out_sb[:])
            else:
                part = pdram.tile([B, DVA], F32, name="part", tag="part")
                with tc.For_i(0, loop_iters, 1):
                    _emit_body(nc, pools, io, mask01, part, stage=stage)
                # dummy output so the NEFF has a valid ExternalOutput write
                out_sb = pep.tile([B_SH, DV], F32, name="out_sb", tag="out_sb")
                nc.vector.memset(out_sb[:], 0.0)
                nc.sync.dma_start(io["out"][:], out_sb[:])
    nc.compile()
    return nc


_CACHE: dict = {}


def _get_nc():
    if "nc" not in _CACHE:
        _CACHE["nc"] = build_nc()
    return _CACHE["nc"]


def make_in_maps(q, k, v, K_cache, V_cache):
    q = np.asarray(q, np.float32)
    k = np.asarray(k, np.float32)
    v = np.asarray(v, np.float32)
    K_cache = np.asarray(K_cache, np.float32)
    V_cache = np.asarray(V_cache, np.float32)

    scale = 1.0 / np.sqrt(np.float32(DK))
    qt = np.ascontiguousarray((q * scale).T)  # [128, 1024]

    in_maps = []
    for c in range(N_CORES):
        Ksh = np.concatenate(
            [K_cache[c * S_SH : (c + 1) * S_SH], k[c * B_SH : (c + 1) * B_SH]],
            axis=0,
        )  # [4224, 128]
        kt = np.ascontiguousarray(Ksh.T)  # [128, 4224]
        Vsh = np.concatenate(
            [V_cache[c * S_SH : (c + 1) * S_SH], v[c * B_SH : (c + 1) * B_SH]],
            axis=0,
        )
        va = np.zeros((NKEY, DVA), np.float32)
        va[:, :DV] = Vsh
        va[:, DV] = 1.0
        va = va.reshape(NT, 128, DVA).astype(ml_dtypes.bfloat16)
        thr = (c * B_SH + np.arange(128, dtype=np.float32)).reshape(128, 1)
        in_maps.append({"kt": kt, "qt": qt, "vaug": va, "thr": thr})
    return in_maps


def kernel(q, k, v, K_cache, V_cache):
    in_maps = make_in_maps(q, k, v, K_cache, V_cache)
    res = run_bass_kernel_spmd(
        _get_nc(), in_maps, core_ids=list(range(N_CORES))
    )
    out = np.concatenate(
        [res.results[c]["out"] for c in range(N_CORES)], axis=0
    )
    return np.ascontiguousarray(out, dtype=np.float32)
